# revision 1
# baseline (speedup 1.0000x reference)
"""DVBF Trainium2 kernel: data-parallel across 8 NeuronCores on the batch axis.

On-device layout: feature-major [feature, sample] (features on SBUF partitions,
samples on the free dim), 64 samples per core.

Phases:
  P1 fw-LSTM (T steps). Gates accumulate in PSUM (Wih·x + b·1 + Whh·h); one
     tanh ACT per step over all four gates; sigmoid(x)=0.5*tanh(0.5x)+0.5 with
     the 0.5 folded into host-preprocessed weights so the whole kernel uses a
     single ACT table set (exp/tanh/relu/log/square all co-resident).
     Host reorders gates to (i,f,o,g) so the sigmoid-fix is one DVE op.
  P2 bw-LSTM single step on x[:,T-1] (reverse-scan last element semantics).
  P3 initial sample MLPs -> w1 -> z1.
  P4 transition recurrence t=1..T-1. MLPs feature-major, stationary weights.
     The mixture contraction runs flipped: stationary = activations
     ([z;(u;w)]), streaming = D^T [224, 2048] -> batch-major Y[64, 2048] in
     PSUM; softmax handled unnormalized (exp via ACT, sum via ones-matmul,
     reciprocal folded into the per-sample mixing scalars); mixing = 16
     scalar_tensor_tensor FMAs with per-partition scalars; PE-transpose back
     to feature-major z.
  P5 observation decode (grouped over 8 timesteps -> 512-wide matmuls) and
     the squared-error/KL reductions -> per-core partial sums.

Host: shard batch, transpose to feature-major, run SPMD on 8 cores, assemble
the scalar loss from per-core partials (the gather step of data parallelism).
"""
import sys

for _p in ("/opt/trn_rl_repo",):
    if _p not in sys.path:
        sys.path.insert(0, _p)

import numpy as np

N_FULL, T, DX, DU, DZ, DW, M, H = 512, 128, 256, 32, 128, 64, 16, 128
NCORES = 8
NB = N_FULL // NCORES          # 64 samples per core
G4 = 4 * H
LOG2PI = 1.8378770664093453

_CACHE = {}


def _install_tilefix():
    """This walrus build accepts only ONE semaphore wait per CTRL
    (Drain/NoOp) instruction; Tile's final drain carries one wait per live
    semaphore. Split the extras across nops."""
    from concourse import mybir
    from concourse.tile import TileContext, ScopedClock

    if getattr(TileContext, "_waitsplit_installed", False):
        return

    def _patched_dab(self, tick_clock, wait_clock):
        nc = self.nc
        drain_inst = nc.sync.drain()
        wait_clock.add_sem_waits(
            drain_inst.ins, ScopedClock({None: tick_clock.global_clock})
        )
        si = drain_inst.ins.sync_info
        if si and si.on_wait and len(si.on_wait) > 1:
            waits = list(si.on_wait)
            si.on_wait = waits[:1]
            for w in waits[1:]:
                nop = nc.sync.nop(hint="waitsplit", nofuse=True)
                nsi = nop.ins.sync_info
                if nsi is None:
                    nop.ins.sync_info = mybir.SyncInfo(on_wait=[w], on_update=[])
                else:
                    nsi.on_wait = [w]
        nc.all_engine_barrier()
        assert self.sems is not None
        popped = nc._tile_sem_poison_stack.pop()
        assert popped is self._sem_poison
        nc.clear_and_free_semaphores(list(self.sems.allocated().values()))
        nc.all_engine_barrier()

    TileContext._drain_and_barrier = _patched_dab
    TileContext._waitsplit_installed = True


def _split_waits(nc, max_waits=1):
    """This walrus build encodes at most one semaphore wait per instruction
    (any opcode). Hoist extra waits onto same-engine NoOps placed directly
    before the instruction."""
    from concourse import mybir

    n_new = 0
    for f in nc.m.functions:
        for bb in f.blocks:
            il = list(bb.instructions)
            out = []
            changed = False
            for ins in il:
                si = ins.sync_info
                if si and si.on_wait and len(si.on_wait) > max_waits:
                    waits = list(si.on_wait)
                    for w in waits[:-max_waits]:
                        nop = mybir.InstNoOp(
                            name=f"I-ws-{n_new}", engine=ins.engine,
                            ins=[], outs=[],
                            sync_info=mybir.SyncInfo(on_wait=[w], on_update=[]),
                            text_hint="waitsplit")
                        n_new += 1
                        out.append(nop)
                    si.on_wait = waits[-max_waits:]
                    changed = True
                out.append(ins)
            if changed:
                bb.instructions = out
    return n_new


INPUT_SPECS = {
    "x_fm": [2, H, T, NB],
    "ones_bf": [1, NB],
    "u_fm": [DU, T, NB],
    "e_fm": [DW, T, NB],
    "eps1_fm": [DW, NB],
    "fw_wih": [2, H, G4],      # gate order (i,f,o,g); i,f,o cols pre-scaled 0.5
    "fw_whh": [H, G4],
    "fw_b": [1, G4],
    "bw_wih": [2, H, G4],
    "bw_b": [1, G4],
    "i1T": [2, H, H],
    "i1b": [H, 1],
    "i2T": [H, 2 * DW],
    "i2b": [2 * DW, 1],
    "z1T": [DW, H],
    "z1b": [H, 1],
    "z2T": [H, DZ],
    "z2b": [DZ, 1],
    "wp1T_x": [2, H, H],
    "wp1T_z": [DZ, H],
    "wp1T_u": [DU, H],
    "wp1b": [H, 1],
    "wp2T": [H, 2 * DW],
    "wp2b": [2 * DW, 1],
    "vp1T_z": [DZ, H],
    "vp1T_u": [DU, H],
    "vp1b": [H, 1],
    "vp2T": [H, M],
    "vp2b": [M, 1],
    "DT_z": [DZ, M * DZ],
    "DT_uw": [DU + DW, M * DZ],
    "ob1T": [DZ, H],
    "ob1b": [H, 1],
    "ob2T": [H, DX],
    "ob2b": [DX, 1],
}


def build_program(nsteps=T):
    _install_tilefix()
    from contextlib import ExitStack
    import concourse.bass as bass
    import concourse.tile as tile
    from concourse import mybir

    f32 = mybir.dt.float32
    bf16 = mybir.dt.bfloat16
    AF = mybir.ActivationFunctionType
    OP = mybir.AluOpType
    AX = mybir.AxisListType

    nc = bass.Bass("TRN2", target_bir_lowering=False, debug=False)

    specs = dict(INPUT_SPECS)
    for k in ("x_fm", "u_fm", "e_fm"):
        specs[k] = specs[k][:-2] + [nsteps, NB]

    BF16_INS = {"x_fm", "ones_bf", "fw_wih", "fw_whh", "fw_b", "bw_wih",
                "bw_b", "i1T", "i2T", "z1T", "z2T", "wp1T_x", "wp2T",
                "vp2T"}
    F32R_INS = {"DT_z", "DT_uw", "ob1T", "ob2T", "wp1T_z", "wp1T_u",
                "vp1T_z", "vp1T_u", "u_fm"}
    f32r = mybir.dt.float32r
    ins = {}
    for name, shape in specs.items():
        dt_ = bf16 if name in BF16_INS else (
            f32r if name in F32R_INS else f32)
        ins[name] = nc.declare_dram_parameter(name, shape, dt_, isOutput=False)
    out_h = nc.declare_dram_parameter("out", [1, 8], f32, isOutput=True)

    ident64 = nc.inline_tensor(np.eye(NB, dtype=np.float32), name="ident64")
    ones16 = nc.inline_tensor(np.ones((M, 1), dtype=np.float32), name="ones16")
    ones64c = nc.inline_tensor(np.ones((NB, 1), dtype=np.float32), name="ones64c")
    ones128c = nc.inline_tensor(np.ones((H, 1), dtype=np.float32), name="ones128c")
    

    with tile.TileContext(nc) as tc, ExitStack() as top:
        W = top.enter_context(tc.tile_pool(name="weights", bufs=1))
        SB = top.enter_context(tc.tile_pool(name="state", bufs=1))
        ACTS = top.enter_context(tc.tile_pool(name="acts", bufs=3))

        def wtile(name):
            shape = specs[name]
            wdt = bf16 if name in BF16_INS else (
                f32r if name in F32R_INS else f32)
            if len(shape) == 3 or shape[0] > 128:
                if len(shape) == 3:
                    nchunk, sub = shape[0], shape[1:]
                    parts = []
                    for c in range(nchunk):
                        t = W.tile(sub, wdt, tag=f"{name}_{c}")
                        nc.sync.dma_start(t[...], ins[name][c])
                        parts.append(t)
                    return parts
                else:
                    assert shape[0] % 128 == 0
                    nchunk, sub = shape[0] // 128, [128] + shape[1:]
                    parts = []
                    for c in range(nchunk):
                        t = W.tile(sub, wdt, tag=f"{name}_{c}")
                        nc.sync.dma_start(t[...],
                                          ins[name][c * 128:(c + 1) * 128])
                        parts.append(t)
                    return parts
            t = W.tile(shape, wdt, tag=name)
            nc.sync.dma_start(t[...], ins[name][...])
            return t

        s = {k: wtile(k) for k in specs
             if k not in ("x_fm", "u_fm", "e_fm", "wp1T_u", "vp1T_u", "ones_bf")}
        # u-part weights live at base partition 64 to match the uw tile slice
        for name in ("wp1T_u", "vp1T_u"):
            t = W.tile([DW + DU, H], f32r, tag=name)
            nc.sync.dma_start(t[DW:DW + DU, :], ins[name][...])
            s[name] = t

        def ctile(handle, shape, tag):
            t = W.tile(shape, f32, tag=tag)
            nc.sync.dma_start(t[...], handle[...])
            return t

        c_id64 = ctile(ident64, [NB, NB], "ident64")
        c_ones16 = ctile(ones16, [M, 1], "ones16")
        c_ones64 = ctile(ones64c, [NB, 1], "ones64c")
        c_ones128 = ctile(ones128c, [H, 1], "ones128c")
        c_ones1x64 = W.tile([1, NB], bf16, tag="ones1x64")
        nc.sync.dma_start(c_ones1x64[...], ins["ones_bf"][...])

        x_t = {}
        for t in range(nsteps):
            for c in range(2):
                xt = SB.tile([H, NB], bf16, tag=f"x{c}_{t}")
                nc.sync.dma_start(xt[...], ins["x_fm"][c, :, t, :])
                x_t[(c, t)] = xt

        z_buf = SB.tile([DZ, nsteps, NB], f32r, tag="z_buf")
        m_buf = SB.tile([DW, nsteps, NB], f32, tag="m_buf")
        s_buf = SB.tile([DW, nsteps, NB], f32, tag="s_buf")

        # ---------------- P1: forward LSTM + P2: backward single step -----
        with ExitStack() as ph:
            PSG = ph.enter_context(tc.tile_pool(name="ps_g", bufs=2, space="PSUM"))
            LST = ph.enter_context(tc.tile_pool(name="lstm_sb", bufs=3))

            def lstm_gates(wih, b, xt0, xt1, h_prev, whh):
                g_ps = PSG.tile([H, 4 * NB], f32, tag="gates")
                for g in range(4):
                    o = g_ps[:, g * NB:(g + 1) * NB]
                    nc.tensor.matmul(o, wih[0][:, g * H:(g + 1) * H], xt0[...],
                                     start=True, stop=False)
                    nc.tensor.matmul(o, wih[1][:, g * H:(g + 1) * H], xt1[...],
                                     start=False, stop=False)
                    nc.tensor.matmul(o, b[:, g * H:(g + 1) * H], c_ones1x64[...],
                                     start=False, stop=(h_prev is None))
                    if h_prev is not None:
                        nc.tensor.matmul(o, whh[:, g * H:(g + 1) * H],
                                         h_prev[...], start=False, stop=True)
                th = LST.tile([H, 4 * NB], f32, tag="tanh_g")
                nc.scalar.activation(th[...], g_ps[...], AF.Tanh)
                sg = LST.tile([H, 3 * NB], f32, tag="sig_g")
                nc.vector.tensor_scalar(sg[...], th[:, 0:3 * NB], 0.5, 0.5,
                                        OP.mult, OP.add)
                return th, sg  # gate order (i,f,o,g): sg = sig(i,f,o)

            h_prev = None
            c_prev = None
            for t in range(nsteps):
                th, sg = lstm_gates(s["fw_wih"], s["fw_b"], x_t[(0, t)],
                                    x_t[(1, t)], h_prev, s["fw_whh"])
                ig = LST.tile([H, NB], f32, tag="ig")
                nc.vector.tensor_mul(ig[...], sg[:, 0:NB], th[:, 3 * NB:4 * NB])
                if c_prev is not None:
                    fc = LST.tile([H, NB], f32, tag="fc")
                    nc.vector.tensor_mul(fc[...], sg[:, NB:2 * NB], c_prev[...])
                    c_new = LST.tile([H, NB], f32, tag="c_st")
                    nc.vector.tensor_add(c_new[...], fc[...], ig[...])
                else:
                    c_new = ig
                tc_ = LST.tile([H, NB], f32, tag="tanh_c")
                nc.scalar.activation(tc_[...], c_new[...], AF.Tanh)
                h_new = LST.tile([H, NB], bf16, tag="h_st")
                nc.vector.tensor_mul(h_new[...], sg[:, 2 * NB:3 * NB], tc_[...])
                h_prev, c_prev = h_new, c_new

            h_fw = SB.tile([H, NB], bf16, tag="h_fw")
            nc.vector.tensor_copy(h_fw[...], h_prev[...])

            thb, sgb = lstm_gates(s["bw_wih"], s["bw_b"], x_t[(0, nsteps - 1)],
                                  x_t[(1, nsteps - 1)], None, None)
            cb = LST.tile([H, NB], f32, tag="ig")
            nc.vector.tensor_mul(cb[...], sgb[:, 0:NB], thb[:, 3 * NB:4 * NB])
            tcb = LST.tile([H, NB], f32, tag="tanh_c")
            nc.scalar.activation(tcb[...], cb[...], AF.Tanh)
            h_bw = SB.tile([H, NB], bf16, tag="h_bw")
            nc.vector.tensor_mul(h_bw[...], sgb[:, 2 * NB:3 * NB], tcb[...])

        # ---------------- P3: initial sample MLPs -------------------------
        with ExitStack() as ph:
            PSS = ph.enter_context(tc.tile_pool(name="ps_init", bufs=2, space="PSUM"))
            p1 = PSS.tile([H, NB], f32, tag="ps_a")
            nc.tensor.matmul(p1[...], s["i1T"][0][...], h_fw[...], start=True, stop=False)
            nc.tensor.matmul(p1[...], s["i1T"][1][...], h_bw[...], start=False, stop=True)
            r1 = ACTS.tile([H, NB], bf16, tag="relu1")
            nc.scalar.activation(r1[...], p1[...], AF.Relu, bias=s["i1b"][...])
            p0 = PSS.tile([2 * DW, NB], f32, tag="ps_b")
            nc.tensor.matmul(p0[...], s["i2T"][...], r1[...], start=True, stop=True)
            nc.vector.tensor_scalar(m_buf[:, 0, :], p0[0:DW, :],
                                    s["i2b"][0:DW, :], None, OP.add)
            nc.scalar.activation(s_buf[:, 0, :], p0[DW:2 * DW, :], AF.Exp,
                                 bias=s["i2b"][DW:2 * DW, :])
            t1 = ACTS.tile([DW, NB], f32, tag="t1w")
            nc.vector.scalar_tensor_tensor(t1[...], s_buf[:, 0, :], 1e-5,
                                           s["eps1_fm"][...], OP.add, OP.mult)
            w1 = ACTS.tile([DW, NB], bf16, tag="w1")
            nc.vector.tensor_add(w1[...], t1[...], m_buf[:, 0, :])
            pz = PSS.tile([H, NB], f32, tag="ps_a")
            nc.tensor.matmul(pz[...], s["z1T"][...], w1[...], start=True, stop=True)
            rz = ACTS.tile([H, NB], bf16, tag="relu1")
            nc.scalar.activation(rz[...], pz[...], AF.Relu, bias=s["z1b"][...])
            pz2 = PSS.tile([DZ, NB], f32, tag="ps_b")
            nc.tensor.matmul(pz2[...], s["z2T"][...], rz[...], start=True, stop=True)
            nc.vector.tensor_scalar(z_buf[:, 0, :], pz2[...], s["z2b"][...],
                                    None, OP.add)

        # ---------------- P4: transition recurrence -----------------------
        with ExitStack() as ph:
            PSY = ph.enter_context(tc.tile_pool(name="ps_y", bufs=1, space="PSUM"))
            PSS = ph.enter_context(tc.tile_pool(name="ps_sm", bufs=4, space="PSUM"))
            TRN = ph.enter_context(tc.tile_pool(name="trn_sb", bufs=3))
            avx = SB.tile([32, NB], f32, tag="avx")
            nc.vector.memset(avx[...], 0.0)
            ebm = SB.tile([NB, 32], f32, tag="ebm")

            for t in range(1, nsteps):
                zp = z_buf[:, t - 1, :]
                uw = TRN.tile([DU + DW, NB], f32r, tag="uw")
                nc.sync.dma_start(uw[DW:DW + DU, :], ins["u_fm"][:, t - 1, :])
                et = TRN.tile([DW, NB], f32, tag="et")
                nc.sync.dma_start(et[...], ins["e_fm"][:, t, :])

                pw1 = PSS.tile([H, NB], f32, tag="ps_sm")
                nc.tensor.matmul(pw1[...], s["wp1T_x"][0][...], x_t[(0, t)][...],
                                 start=True, stop=False)
                nc.tensor.matmul(pw1[...], s["wp1T_x"][1][...], x_t[(1, t)][...],
                                 start=False, stop=False)
                nc.tensor.matmul(pw1[...], s["wp1T_z"][...], zp,
                                 start=False, stop=False)
                nc.tensor.matmul(pw1[...], s["wp1T_u"][DW:DW + DU, :], uw[DW:DW + DU, :],
                                 start=False, stop=True)
                th1 = TRN.tile([H, NB], bf16, tag="th1")
                nc.scalar.activation(th1[...], pw1[...], AF.Tanh,
                                     bias=s["wp1b"][...])
                pw = PSS.tile([2 * DW, NB], f32, tag="ps_sm")
                nc.tensor.matmul(pw[...], s["wp2T"][...], th1[...],
                                 start=True, stop=True)
                nc.vector.tensor_scalar(m_buf[:, t, :], pw[0:DW, :],
                                        s["wp2b"][0:DW, :], None, OP.add)
                nc.scalar.activation(s_buf[:, t, :], pw[DW:2 * DW, :], AF.Exp,
                                     bias=s["wp2b"][DW:2 * DW, :])
                t1w = TRN.tile([DW, NB], f32, tag="t1w")
                nc.vector.scalar_tensor_tensor(t1w[...], s_buf[:, t, :], 0.01,
                                               et[...], OP.add, OP.mult)
                nc.vector.tensor_add(uw[0:DW, :], t1w[...], m_buf[:, t, :])

                pv1 = PSS.tile([H, NB], f32, tag="ps_sm")
                nc.tensor.matmul(pv1[...], s["vp1T_z"][...], zp,
                                 start=True, stop=False)
                nc.tensor.matmul(pv1[...], s["vp1T_u"][DW:DW + DU, :], uw[DW:DW + DU, :],
                                 start=False, stop=True)
                rv = TRN.tile([H, NB], bf16, tag="rv")
                nc.scalar.activation(rv[...], pv1[...], AF.Relu,
                                     bias=s["vp1b"][...])
                av = PSS.tile([M, NB], f32, tag="ps_sm")
                nc.tensor.matmul(av[...], s["vp2T"][...], rv[...],
                                 start=True, stop=True)
                nc.scalar.activation(avx[0:M, :], av[...], AF.Exp,
                                     bias=s["vp2b"][...])
                nc.vector.transpose(ebm[0:32, 0:32], avx[0:32, 0:32])
                nc.vector.transpose(ebm[32:64, 0:32], avx[0:32, 32:64])
                rsum = TRN.tile([NB, 1], f32, tag="rsum")
                nc.vector.tensor_reduce(rsum[...], ebm[:, 0:M], AX.X, OP.add)
                rec = TRN.tile([NB, 1], f32, tag="rec")
                nc.vector.reciprocal(rec[...], rsum[...])
                ehat = TRN.tile([NB, M], f32, tag="ehat")
                nc.vector.tensor_scalar(ehat[...], ebm[:, 0:M], rec[...],
                                        None, OP.mult)

                ybm = PSY.tile([NB, M * DZ], f32, tag="ybm")
                for q in range(4):
                    sl = slice(q * 512, (q + 1) * 512)
                    nc.tensor.matmul(ybm[:, sl], zp, s["DT_z"][:, sl],
                                     start=True, stop=False)
                    nc.tensor.matmul(ybm[:, sl], uw[...], s["DT_uw"][:, sl],
                                     start=False, stop=True)

                zacc0 = TRN.tile([NB, DZ], f32, tag="zbm0", name="zacc0")
                zacc1 = TRN.tile([NB, DZ], f32, tag="zbm1", name="zacc1")
                zacc = [zacc0, zacc1]
                nc.vector.tensor_scalar(zacc[0][...], ybm[:, 0:DZ],
                                        ehat[:, 0:1], None, OP.mult)
                for m in range(1, M):
                    nc.vector.scalar_tensor_tensor(
                        zacc[m % 2][...], ybm[:, m * DZ:(m + 1) * DZ],
                        ehat[:, m:m + 1], zacc[(m - 1) % 2][...],
                        OP.mult, OP.add)
                ztr = PSS.tile([DZ, NB], f32, tag="ps_sm")
                nc.tensor.transpose(ztr[...], zacc[(M - 1) % 2][...],
                                    c_id64[...])
                nc.vector.tensor_copy(z_buf[:, t, :], ztr[...])

        # ---------------- P5: decode + reductions -------------------------
        with ExitStack() as ph:
            PSD = ph.enter_context(tc.tile_pool(name="ps_dec", bufs=3, space="PSUM"))
            PSF = ph.enter_context(tc.tile_pool(name="ps_fin", bufs=1, space="PSUM"))
            DEC = ph.enter_context(tc.tile_pool(name="dec_sb", bufs=2))
            TG = 8                     # timesteps per decode group
            NGRP = nsteps // TG if nsteps % TG == 0 else (nsteps + TG - 1) // TG
            FD = TG * NB
            sqacc = SB.tile([H, 2 * NGRP], f32, tag="sqacc")
            for g in range(NGRP):
                t0g, t1g = g * TG, min((g + 1) * TG, nsteps)
                fd = (t1g - t0g) * NB
                hob_ps = PSD.tile([H, FD], f32, tag="dec_ps")
                nc.tensor.matmul(hob_ps[:, 0:fd], s["ob1T"][...],
                                 z_buf[:, t0g:t1g, :], start=True, stop=True)
                hob = DEC.tile([H, FD], f32r, tag="hob")
                nc.scalar.activation(hob[:, 0:fd], hob_ps[:, 0:fd], AF.Relu,
                                     bias=s["ob1b"][...])
                for c in range(2):
                    xg = DEC.tile([H, FD], bf16, tag="xg")
                    nc.sync.dma_start(xg[:, 0:fd],
                                      ins["x_fm"][c, :, t0g:t1g, :])
                    xr = PSD.tile([H, FD], f32, tag="dec_ps")
                    nc.tensor.matmul(xr[:, 0:fd],
                                     s["ob2T"][:, c * H:(c + 1) * H],
                                     hob[:, 0:fd], start=True, stop=True)
                    df = DEC.tile([H, FD], f32, tag="df")
                    nc.vector.scalar_tensor_tensor(
                        df[:, 0:fd], xr[:, 0:fd], s["ob2b"][c][...],
                        xg[:, 0:fd], OP.add, OP.subtract)
                    sq = DEC.tile([H, FD], f32, tag="sq")
                    nc.scalar.activation(
                        sq[:, 0:fd], df[:, 0:fd], AF.Square,
                        accum_out=sqacc[:, 2 * g + c:2 * g + c + 1])

            # KL reductions, chunked over timesteps
            KCH = 16                   # steps per chunk
            kcols = []
            kl_s = SB.tile([DW, 64], f32, tag="kl_s")
            col = 0
            t0k = 1
            while t0k < nsteps:
                t1k = min(t0k + KCH, nsteps)
                fd = (t1k - t0k) * NB
                sf = DEC.tile([DW, KCH * NB], f32, tag="sf")
                nc.vector.tensor_scalar(sf[:, 0:fd], s_buf[:, t0k:t1k, :],
                                        0.01, None, OP.add, OP.add,
                                        accum_out=kl_s[:, col:col + 1])
                lg = DEC.tile([DW, KCH * NB], f32, tag="lgk")
                nc.scalar.activation(lg[:, 0:fd], sf[:, 0:fd], AF.Ln,
                                     accum_out=kl_s[:, col + 1:col + 2])
                m2 = DEC.tile([DW, KCH * NB], f32, tag="m2k")
                nc.scalar.activation(m2[:, 0:fd], m_buf[:, t0k:t1k, :],
                                     AF.Square,
                                     accum_out=kl_s[:, col + 2:col + 3])
                kcols.append(col)
                col += 3
                t0k = t1k
            # t = 0 (epsilon 1e-5)
            s0f = DEC.tile([DW, NB], f32, tag="s0f")
            nc.vector.tensor_scalar(s0f[...], s_buf[:, 0, :], 1e-5, None,
                                    OP.add, OP.add,
                                    accum_out=kl_s[:, col:col + 1])
            lg0 = DEC.tile([DW, NB], f32, tag="lg0")
            nc.scalar.activation(lg0[...], s0f[...], AF.Ln,
                                 accum_out=kl_s[:, col + 1:col + 2])
            m20 = DEC.tile([DW, NB], f32, tag="m20")
            nc.scalar.activation(m20[...], m_buf[:, 0, :], AF.Square,
                                 accum_out=kl_s[:, col + 2:col + 3])
            ncols = col + 3

            # per-partition: sum(s) + sum(m^2) - sum(log s) across all chunks
            spm = DEC.tile([DW, ncols], f32, tag="spm")
            nc.vector.tensor_scalar(spm[:, 0:ncols], kl_s[:, 0:ncols],
                                    1.0, None, OP.mult)
            # negate the log columns then reduce everything
            for c0 in list(kcols) + [col]:
                nc.vector.tensor_scalar(spm[:, c0 + 1:c0 + 2],
                                        kl_s[:, c0 + 1:c0 + 2], -1.0, None,
                                        OP.mult)
            klred = DEC.tile([DW, 1], f32, tag="klred")
            nc.vector.tensor_reduce(klred[...], spm[:, 0:ncols], AX.X, OP.add)
            sqred = DEC.tile([H, 1], f32, tag="sqred")
            nc.vector.tensor_reduce(sqred[...], sqacc[:, 0:2 * NGRP], AX.X,
                                    OP.add)

            fin = PSF.tile([1, 2], f32, tag="fin")
            nc.tensor.matmul(fin[:, 0:1], sqred[...], c_ones128[...],
                             start=True, stop=True)
            nc.tensor.matmul(fin[:, 1:2], klred[...], c_ones64[...],
                             start=True, stop=True)
            outt = DEC.tile([1, 8], f32, tag="outt")
            nc.vector.memset(outt[...], 0.0)
            nc.vector.tensor_copy(outt[:, 0:2], fin[...])
            nc.sync.dma_start(out_h[...], outt[...])

    _split_waits(nc)
    return nc, specs


def preprocess(inputs, nsteps=T):
    """Shard + feature-major transpose + weight prep. Returns list of 8
    per-core input maps."""
    import ml_dtypes
    f = np.float32
    bf = ml_dtypes.bfloat16
    x = inputs["x"]
    u = inputs["u"]
    eps = inputs["eps"]
    eps1 = inputs["eps1"]

    def gate_prep(wih, whh, b):
        # reference gate order (i,f,g,o) -> ours (i,f,o,g); 0.5-scale i,f,o
        def perm(a, axis=0):
            blocks = np.split(a, 4, axis=axis)
            i, fo, g, o = blocks
            return [i, fo, o, g]

        sc = np.array([0.5, 0.5, 0.5, 1.0], dtype=f)

        def scale_cat(blocks):
            return np.concatenate([bl * sc[k] for k, bl in enumerate(blocks)],
                                  axis=0)

        wih_p = scale_cat(perm(wih))         # [4H, DX]
        b_p = scale_cat(perm(b))             # [4H]
        whh_p = scale_cat(perm(whh)) if whh is not None else None
        # lhsT chunks: Wih^T [DX, 4H] split into 2 x [H, 4H]
        wihT = np.ascontiguousarray(wih_p.T.reshape(2, H, G4))
        whhT = np.ascontiguousarray(whh_p.T) if whh_p is not None else None
        return wihT, whhT, np.ascontiguousarray(b_p[None, :])

    fw_wihT, fw_whhT, fw_bP = gate_prep(inputs["fw_Wih"], inputs["fw_Whh"],
                                        inputs["fw_b"])
    bw_wihT, _, bw_bP = gate_prep(inputs["bw_Wih"], None, inputs["bw_b"])

    A, B, C = inputs["A"], inputs["B"], inputs["C"]
    # DT_z[j, m*DZ+i] = A[m, i, j]; DT_uw rows: u-part B, w-part C
    DT_z = np.ascontiguousarray(
        A.transpose(2, 0, 1).reshape(DZ, M * DZ)).astype(f)
    DT_u = B.transpose(2, 0, 1).reshape(DU, M * DZ)
    DT_w = C.transpose(2, 0, 1).reshape(DW, M * DZ)
    DT_uw = np.ascontiguousarray(np.concatenate([DT_w, DT_u], axis=0)).astype(f)

    def col(a):
        return np.ascontiguousarray(a.astype(f)[:, None])

    common = {
        "ones_bf": np.ones((1, NB), dtype=bf),
        "fw_wih": fw_wihT.astype(bf), "fw_whh": fw_whhT.astype(bf),
        "fw_b": fw_bP.astype(bf),
        "bw_wih": bw_wihT.astype(bf), "bw_b": bw_bP.astype(bf),
        "i1T": np.ascontiguousarray(
            inputs["i1_w"].T.reshape(2, H, H)).astype(bf),
        "i1b": col(inputs["i1_b"]),
        "i2T": np.ascontiguousarray(inputs["i2_w"].T).astype(bf),
        "i2b": col(inputs["i2_b"]),
        "z1T": np.ascontiguousarray(inputs["z1_w"].T).astype(bf),
        "z1b": col(inputs["z1_b"]),
        "z2T": np.ascontiguousarray(inputs["z2_w"].T).astype(bf),
        "z2b": col(inputs["z2_b"]),
        "wp1T_x": np.ascontiguousarray(
            inputs["wp_w1"][:, 0:DX].T.reshape(2, H, H)).astype(bf),
        "wp1T_z": np.ascontiguousarray(
            inputs["wp_w1"][:, DX:DX + DZ].T).astype(f),
        "wp1T_u": np.ascontiguousarray(
            inputs["wp_w1"][:, DX + DZ:DX + DZ + DU].T).astype(f),
        "wp1b": col(inputs["wp_b1"]),
        "wp2T": np.ascontiguousarray(inputs["wp_w2"].T).astype(bf),
        "wp2b": col(inputs["wp_b2"]),
        "vp1T_z": np.ascontiguousarray(inputs["vp_w1"][:, 0:DZ].T).astype(f),
        "vp1T_u": np.ascontiguousarray(
            inputs["vp_w1"][:, DZ:DZ + DU].T).astype(f),
        "vp1b": col(inputs["vp_b1"]),
        "vp2T": np.ascontiguousarray(inputs["vp_w2"].T).astype(bf),
        "vp2b": col(inputs["vp_b2"]),
        "DT_z": DT_z, "DT_uw": DT_uw,
        "ob1T": np.ascontiguousarray(inputs["ob_w1"].T).astype(f),
        "ob1b": col(inputs["ob_b1"]),
        "ob2T": np.ascontiguousarray(inputs["ob_w2"].T).astype(f),
        "ob2b": col(inputs["ob_b2"]),
    }

    maps = []
    for ci in range(NCORES):
        sl = slice(ci * NB, (ci + 1) * NB)
        xs = x[sl, :nsteps]                       # [NB, t, DX]
        m = dict(common)
        m["x_fm"] = np.ascontiguousarray(
            xs.transpose(2, 1, 0).reshape(2, H, nsteps, NB)).astype(bf)
        m["u_fm"] = np.ascontiguousarray(
            u[sl, :nsteps].transpose(2, 1, 0)).astype(f)
        m["e_fm"] = np.ascontiguousarray(
            eps[sl, :nsteps].transpose(2, 1, 0)).astype(f)
        m["eps1_fm"] = np.ascontiguousarray(eps1[sl].T).astype(f)
        maps.append(m)
    return maps


def run(inputs, nsteps=T, trace=False, reps=1):
    import time
    from concourse.bass_utils import run_bass_kernel_spmd

    key = nsteps
    if key not in _CACHE:
        _CACHE[key] = build_program(nsteps)
    nc, _specs = _CACHE[key]
    maps = preprocess(inputs, nsteps)
    walls = []
    res = None
    for _ in range(max(1, reps)):
        t0 = time.perf_counter()
        res = run_bass_kernel_spmd(nc, maps, list(range(NCORES)), trace=trace)
        walls.append(time.perf_counter() - t0)
    res.exec_walls = walls
    S1 = 0.0
    SKL = 0.0
    for ci in range(NCORES):
        o = res.results[ci]["out"]
        S1 += float(o[0, 0])
        SKL += float(o[0, 1])
    n, t_, dx, dw = N_FULL, nsteps, DX, DW
    logprob = -0.5 * S1 - 0.5 * n * t_ * dx * LOG2PI
    kl = 0.5 * (SKL - n * t_ * dw)
    loss = -(logprob - kl)
    return np.float32(loss), res


def kernel(**inputs):
    loss, _res = run(inputs, T, trace=False)
    return np.asarray(loss, dtype=np.float32)


def run_null(inputs, nsteps=T, reps=3):
    """Same inputs/outputs, trivial body: isolates upload/dispatch overhead."""
    import time
    from contextlib import ExitStack
    import concourse.bass as bass
    import concourse.tile as tile
    from concourse import mybir
    from concourse.bass_utils import run_bass_kernel_spmd

    _install_tilefix()
    key = ("null", nsteps)
    if key not in _CACHE:
        f32 = mybir.dt.float32
        bf16 = mybir.dt.bfloat16
        f32r = mybir.dt.float32r
        nc = bass.Bass("TRN2", target_bir_lowering=False, debug=False)
        specs = dict(INPUT_SPECS)
        for k in ("x_fm", "u_fm", "e_fm"):
            specs[k] = specs[k][:-2] + [nsteps, NB]
        BF16_INS = {"x_fm", "ones_bf", "fw_wih", "fw_whh", "fw_b", "bw_wih",
                    "bw_b", "i1T", "i2T", "z1T", "z2T", "wp1T_x", "wp2T",
                    "vp2T"}
        F32R_INS = {"DT_z", "DT_uw", "ob1T", "ob2T", "wp1T_z", "wp1T_u",
                    "vp1T_z", "vp1T_u", "u_fm"}
        ins = {}
        for name, shape in specs.items():
            dt_ = bf16 if name in BF16_INS else (
                f32r if name in F32R_INS else f32)
            ins[name] = nc.declare_dram_parameter(name, shape, dt_,
                                                  isOutput=False)
        out_h = nc.declare_dram_parameter("out", [1, 8], f32, isOutput=True)
        with tile.TileContext(nc) as tc, ExitStack() as top:
            P = top.enter_context(tc.tile_pool(name="p", bufs=1))
            t = P.tile([1, 8], f32, tag="t")
            nc.sync.dma_start(t[...], ins["eps1_fm"][0:1, 0:8])
            nc.sync.dma_start(out_h[...], t[...])
        _split_waits(nc)
        _CACHE[key] = (nc, specs)
    nc, _specs = _CACHE[key]
    maps = preprocess(inputs, nsteps)
    walls = []
    for _ in range(max(1, reps)):
        t0 = time.perf_counter()
        run_bass_kernel_spmd(nc, maps, list(range(NCORES)))
        walls.append(time.perf_counter() - t0)
    return walls

